# revision 1
# baseline (speedup 1.0000x reference)
"""Trainium2 Bass kernel for nn_DeepLatent loss (chamfer + L2 of a per-point MLP).

Strategy (8 cores, data-parallel over batch B=32 -> 4 samples/core):
  Per core, per sample s (channel-major layout: activations stored [C, Npoints]):
    h1 = relu(W1o.T @ obs^T + latbias)        latbias = W1lat.T @ latent + b1 (tiny matmul)
    h2 = relu(W2.T @ h1 + b2)
    h3 = relu(W3.T @ h2 + b3)
    delta = W4.T @ h3                         est = obs + delta + b4
  Chamfer via augmented grams (K=6 matmuls: 3 coord rows + 3 aux rows):
    G [n,m]  = gt_n . est_m - |est_m|^2/2     (aux lhsT rows = -0.5, aux rhs rows = est^2)
    G'[m,n]  = est_m . gt_n - |gt_n|^2/2
    min_m d2[n,m] = |gt_n|^2 - 2 max_m G[n,m]   (max via fused DVE tensor_tensor_reduce)
  Per-core partial sums (max-sums, sq-sums, cross-sum) are combined on the host.

All matmuls use float32r (fp22 truncation, 1 cycle/col at free-dim>=256).
"""

import ml_dtypes
import numpy as np
from contextlib import ExitStack

import concourse.bass as bass
import concourse.bacc as bacc
import concourse.mybir as mybir
import concourse.tile as tile
from concourse.bass_utils import run_bass_kernel_spmd

F32 = mybir.dt.float32
F32R = mybir.dt.float32r
BF16 = mybir.dt.bfloat16
AX = mybir.AxisListType
OP = mybir.AluOpType
ACTF = mybir.ActivationFunctionType

B, N, L = 32, 1024, 256
NCORES = 8
BS = B // NCORES  # samples per core
NT = N // 128     # n-tiles per sample
NEG = -3.0e38

# test.py hooks
TRACE = False
LAST = None


def _r(ap):
    return ap.bitcast(F32R)


def build_program(do_mlp=True, do_gram=True):
    nc = bacc.Bacc()

    # host-pretransposed layouts: every DMA below is inner-contiguous
    obs_d = nc.dram_tensor("obs_t", [3, BS, N], BF16, kind="ExternalInput")[:]
    gt_d = nc.dram_tensor("gt_t", [3, BS, N], BF16, kind="ExternalInput")[:]
    lat_d = nc.dram_tensor("lat_t", [L, BS], F32, kind="ExternalInput")[:]
    W1od = nc.dram_tensor("w1o", [3, 512], BF16, kind="ExternalInput")[:]
    eye3d = nc.dram_tensor("eye3", [3, 3], BF16, kind="ExternalInput")[:]
    W1ld = nc.dram_tensor("w1l", [128, 2, 512], F32, kind="ExternalInput")[:]
    b1d = nc.dram_tensor("b1r", [1, 512], F32, kind="ExternalInput")[:]
    W2d = nc.dram_tensor("w2p", [128, 4, 512], BF16, kind="ExternalInput")[:]
    b2d = nc.dram_tensor("b2p", [128, 4], F32, kind="ExternalInput")[:]
    W3d = nc.dram_tensor("w3p", [128, 4, 256], BF16, kind="ExternalInput")[:]
    b3d = nc.dram_tensor("b3p", [128, 2], F32, kind="ExternalInput")[:]
    W4d = nc.dram_tensor("w4p", [128, 2, 3], BF16, kind="ExternalInput")[:]
    b4d = nc.dram_tensor("b4p", [3, 1], F32, kind="ExternalInput")[:]
    out_d = nc.dram_tensor("partials", [1, 8], F32, kind="ExternalOutput")[:]

    with tile.TileContext(nc) as tc, ExitStack() as ctx:
        singles = ctx.enter_context(tc.tile_pool(name="singles", bufs=1))

        def fixed(shape, name, dtype=F32):
            return singles.tile(shape, dtype, tag=name, name=name)

        # ---------- fixed tiles ----------
        w1o = fixed([3, 512], "w1o", BF16)
        eye3 = fixed([3, 3], "eye3", BF16)
        w1l = fixed([128, 2, 512], "w1l")
        b1r = fixed([1, 512], "b1r")
        w2t = fixed([128, 4, 512], "w2t", BF16)
        w3t = fixed([128, 4, 256], "w3t", BF16)
        w4t = fixed([128, 2, 3], "w4t", BF16)
        b2t = fixed([128, 4], "b2t")
        b3t = fixed([128, 2], "b3t")
        b4p = fixed([3, 1], "b4p")
        latT = fixed([128, 2, BS], "latT")
        ones_r = fixed([1, BS], "ones_r")
        ones_c = fixed([128, 1], "ones_c")
        latb = fixed([128, 4, BS], "latb")
        Pg = fixed([3, BS, N], "Pg", BF16)
        Pe = fixed([3, BS, N], "Pe", BF16)
        Pg2s = fixed([3, N], "Pg2s", BF16)
        Pe2s = fixed([3, N], "Pe2s", BF16)
        SGS = fixed([3, BS], "SGS")
        neghalf = fixed([3, N], "neghalf", BF16)
        M1 = fixed([128, BS * NT], "M1")
        M2 = fixed([128, BS * NT], "M2")
        Ft = fixed([128, 8], "Ft")
        SES = fixed([3, BS], "SES")
        ttr_dump = fixed([128, 512], "ttr_dump")
        outs = fixed([1, 8], "outs")
        A_ = [fixed([128, N], f"Areg{i}", BF16) for i in range(2)]
        B_ = [fixed([128, N], f"Breg{i}", BF16) for i in range(2)]
        C_ = [fixed([128, N], f"Creg{i}", BF16) for i in range(2)]
        D_ = [fixed([128, N], f"Dreg{i}", BF16) for i in range(2)]

        h1p = ctx.enter_context(tc.tile_pool(name="h1", bufs=2))
        h2p = ctx.enter_context(tc.tile_pool(name="h2", bufs=2))
        h3p = ctx.enter_context(tc.tile_pool(name="h3", bufs=2))
        otp = ctx.enter_context(tc.tile_pool(name="obsT", bufs=2))
        psA = ctx.enter_context(tc.tile_pool(name="psA", bufs=2, space="PSUM"))
        psG = ctx.enter_context(tc.tile_pool(name="psG", bufs=2, space="PSUM"))

        # ---------- startup ----------
        nc.sync.dma_start(out=w1o, in_=W1od)
        nc.sync.dma_start(out=eye3, in_=eye3d)
        for k in range(2):
            nc.sync.dma_start(out=latT[:, k, :], in_=lat_d[128 * k:128 * (k + 1), :])
        nc.sync.dma_start(out=b1r, in_=b1d)
        nc.sync.dma_start(out=w1l, in_=W1ld)
        nc.sync.dma_start(out=b2t, in_=b2d)
        nc.sync.dma_start(out=b3t, in_=b3d)
        nc.sync.dma_start(out=b4p, in_=b4d)
        nc.sync.dma_start(out=Pg, in_=gt_d)
        nc.sync.dma_start(out=w2t, in_=W2d)
        nc.sync.dma_start(out=w3t, in_=W3d)
        nc.sync.dma_start(out=w4t, in_=W4d)
        nc.vector.memset(ones_r, 1.0)
        nc.vector.memset(ones_c, 1.0)
        nc.vector.memset(Ft, 0.0)
        # aux lhsT rows {3-5, 35-37} of A/B must be -0.5: memset an fp32
        # staging row-band, then DMA it in (f32r-tagged) since compute engines
        # cannot emit float32r directly.
        nc.vector.memset(neghalf, -0.5)
        for t_ in A_ + B_:
            for g in range(2):
                nc.gpsimd.dma_start(out=t_[32 * g + 3:32 * g + 6, :],
                                    in_=neghalf[:, :])

        # latent bias vectors: latb[cout, c-tile, s] = (latent @ W1[3:] + b1)^T
        for c in range(4):
            lps = psG.tile([128, 1024], F32, tag="g", name=f"latps{c}")
            for k in range(2):
                nc.tensor.matmul(lps[:, 0:BS], w1l[:, k, 128 * c:128 * (c + 1)],
                                 latT[:, k, :], start=(k == 0), stop=False)
            nc.tensor.matmul(lps[:, 0:BS], b1r[:, 128 * c:128 * (c + 1)],
                             ones_r[:, :], start=False, stop=True)
            nc.vector.tensor_copy(latb[:, c, :], lps[:, 0:BS])

        # ---------- per-sample gram rounds (generator; interleaved with next MLP) ----------
        def gram_rounds(s):
            Ar, Br, Cr, Dr = A_[s % 2], B_[s % 2], C_[s % 2], D_[s % 2]
            for lhs_reg, rhs_reg, Mt in ((Ar, Cr, M1), (Br, Dr, M2)):
                for r in range(4):
                    gtiles = []
                    for g in range(2):
                        t = 2 * r + g
                        gp = psG.tile([128, 1024], F32, tag="g", name=f"gp{s}_{r}_{g}")
                        for j in range(2):
                            nc.tensor.matmul(
                                gp[:, 512 * j:512 * (j + 1)],
                                lhs_reg[32 * g:32 * g + 6, 128 * t:128 * (t + 1)],
                                rhs_reg[32 * g:32 * g + 6, 512 * j:512 * (j + 1)],
                                start=True, stop=True)
                        gtiles.append((t, gp))
                    for t, gp in gtiles:
                        nc.vector.tensor_reduce(
                            out=Mt[:, NT * s + t:NT * s + t + 1], in_=gp[:, :],
                            axis=AX.X, op=OP.max)
                    yield

        def advance(it):
            if it is not None:
                next(it, None)

        # ---------- per-sample MLP ----------
        def mlp(s, hooks):
            obsT = otp.tile([3, N], BF16, tag="obsT", name=f"obsT{s}")
            nc.gpsimd.dma_start(out=obsT, in_=obs_d[:, s, :])
            nc.scalar.activation(Pg2s[:, :], Pg[:, s, :], ACTF.Square,
                                 accum_out=SGS[:, s:s + 1])
            Ar, Dr = A_[s % 2], D_[s % 2]
            for g in range(2):
                nc.gpsimd.dma_start(out=Ar[32 * g:32 * g + 3, :], in_=Pg[:, s, :])
                nc.gpsimd.dma_start(out=Dr[32 * g:32 * g + 3, :], in_=Pg[:, s, :])
                nc.gpsimd.dma_start(out=Dr[32 * g + 3:32 * g + 6, :], in_=Pg2s[:, :])

            if not do_mlp:
                # est := gt (copies exercise the same f32r-output DVE/ACT path)
                nc.vector.scalar_tensor_tensor(out=Pe[:, s, :], in0=Pg[:, s, :],
                                               scalar=0.0, in1=Pg[:, s, :],
                                               op0=OP.add, op1=OP.bypass)
                nc.scalar.activation(Pe2s[:, :], Pe[:, s, :], ACTF.Square,
                                     accum_out=SES[:, s:s + 1])
                Br0, Cr0 = B_[s % 2], C_[s % 2]
                for g in range(2):
                    nc.gpsimd.dma_start(out=Br0[32 * g:32 * g + 3, :], in_=Pe[:, s, :])
                    nc.gpsimd.dma_start(out=Cr0[32 * g:32 * g + 3, :], in_=Pe[:, s, :])
                    nc.gpsimd.dma_start(out=Cr0[32 * g + 3:32 * g + 6, :], in_=Pe2s[:, :])
                for _ in range(9):
                    advance(hooks)
                return
            h1t = h1p.tile([128, 4, N], BF16, tag="h1", name=f"h1_{s}")
            for c in range(4):
                ps = psA.tile([128, N], F32, tag="a", name=f"l1ps{s}_{c}")
                for j in range(2):
                    nc.tensor.matmul(ps[:, 512 * j:512 * (j + 1)],
                                     w1o[:, 128 * c:128 * (c + 1)],
                                     obsT[:, 512 * j:512 * (j + 1)],
                                     start=True, stop=True)
                nc.scalar.activation(h1t[:, c, :], ps[:, :], ACTF.Relu,
                                     bias=latb[:, c, s:s + 1])
                advance(hooks)

            h2t = h2p.tile([128, 4, N], BF16, tag="h2", name=f"h2_{s}")
            for c in range(4):
                ps = psA.tile([128, N], F32, tag="a", name=f"l2ps{s}_{c}")
                for j in range(2):
                    for k in range(4):
                        nc.tensor.matmul(ps[:, 512 * j:512 * (j + 1)],
                                         w2t[:, k, 128 * c:128 * (c + 1)],
                                         h1t[:, k, 512 * j:512 * (j + 1)],
                                         start=(k == 0), stop=(k == 3))
                nc.scalar.activation(h2t[:, c, :], ps[:, :], ACTF.Relu,
                                     bias=b2t[:, c:c + 1])
                advance(hooks)

            h3t = h3p.tile([128, 2, N], BF16, tag="h3", name=f"h3_{s}")
            for c in range(2):
                ps = psA.tile([128, N], F32, tag="a", name=f"l3ps{s}_{c}")
                for j in range(2):
                    for k in range(4):
                        nc.tensor.matmul(ps[:, 512 * j:512 * (j + 1)],
                                         w3t[:, k, 128 * c:128 * (c + 1)],
                                         h2t[:, k, 512 * j:512 * (j + 1)],
                                         start=(k == 0), stop=(k == 3))
                nc.scalar.activation(h3t[:, c, :], ps[:, :], ACTF.Relu,
                                     bias=b3t[:, c:c + 1])
                advance(hooks)

            ps4 = psG.tile([128, 1024], F32, tag="g", name=f"l4ps{s}")
            for j in range(2):
                for k in range(2):
                    nc.tensor.matmul(ps4[0:3, 512 * j:512 * (j + 1)],
                                     w4t[:, k, :],
                                     h3t[:, k, 512 * j:512 * (j + 1)],
                                     start=(k == 0), stop=False)
                # obs folded into the same accumulation via identity rows
                nc.tensor.matmul(ps4[0:3, 512 * j:512 * (j + 1)],
                                 eye3[:, :],
                                 obsT[:, 512 * j:512 * (j + 1)],
                                 start=False, stop=True)
            nc.scalar.activation(Pe[:, s, :], ps4[0:3, :], ACTF.Identity,
                                 bias=b4p[:, 0:1])
            nc.scalar.activation(Pe2s[:, :], Pe[:, s, :], ACTF.Square,
                                 accum_out=SES[:, s:s + 1])
            Br, Cr = B_[s % 2], C_[s % 2]
            for g in range(2):
                nc.gpsimd.dma_start(out=Br[32 * g:32 * g + 3, :], in_=Pe[:, s, :])
                nc.gpsimd.dma_start(out=Cr[32 * g:32 * g + 3, :], in_=Pe[:, s, :])
                nc.gpsimd.dma_start(out=Cr[32 * g + 3:32 * g + 6, :], in_=Pe2s[:, :])
            advance(hooks)

        pending = None
        for s in range(BS):
            mlp(s, pending)
            if pending is not None:
                for _ in pending:
                    pass
            if do_gram:
                pending = gram_rounds(s)
        if pending is not None:
            for _ in pending:
                pass

        # ---------- finale ----------
        # cross term sum(gt*est) per coordinate -> Ft col 4 (in-place dump into Pe)
        nc.vector.scalar_tensor_tensor(out=Pe[:, :, :], in0=Pg[:, :, :],
                                       scalar=0.0, in1=Pe[:, :, :],
                                       op0=OP.add, op1=OP.mult,
                                       accum_out=Ft[0:3, 4:5])
        nc.vector.tensor_reduce(out=Ft[0:3, 2:3], in_=SGS[:, :], axis=AX.X, op=OP.add)
        nc.vector.tensor_reduce(out=Ft[0:3, 3:4], in_=SES[:, :], axis=AX.X, op=OP.add)
        nc.vector.tensor_reduce(out=Ft[:, 0:1], in_=M1[:, :], axis=AX.X, op=OP.add)
        nc.vector.tensor_reduce(out=Ft[:, 1:2], in_=M2[:, :], axis=AX.X, op=OP.add)

        fps = psG.tile([128, 1024], F32, tag="g", name="fps")
        nc.tensor.matmul(fps[0:1, 0:8], ones_c[:, :], Ft[:, :],
                         start=True, stop=True)
        nc.scalar.activation(outs[:, :], fps[0:1, 0:8], ACTF.Copy)
        nc.sync.dma_start(out=out_d, in_=outs)

    nc.compile()
    return nc


_program_cache = []


def kernel(**inputs):
    global LAST
    if not _program_cache:
        _program_cache.append(build_program())
    nc = _program_cache[0]

    def f32(x):
        return np.ascontiguousarray(np.asarray(x, dtype=np.float32))

    W1 = np.asarray(inputs["W1"], np.float32)
    W2 = np.asarray(inputs["W2"], np.float32)
    W3 = np.asarray(inputs["W3"], np.float32)
    W4 = np.asarray(inputs["W4"], np.float32)
    shared = {
        "w1o": np.ascontiguousarray(W1[0:3, :].astype(ml_dtypes.bfloat16)),
        "eye3": np.eye(3, dtype=ml_dtypes.bfloat16),
        "w1l": f32(W1[3:259, :].reshape(2, 128, 512).transpose(1, 0, 2)),
        "b1r": f32(np.asarray(inputs["b1"], np.float32).reshape(1, 512)),
        "w2p": np.ascontiguousarray(W2.reshape(4, 128, 512).transpose(1, 0, 2).astype(ml_dtypes.bfloat16)),
        "b2p": f32(np.asarray(inputs["b2"], np.float32).reshape(4, 128).T),
        "w3p": np.ascontiguousarray(W3.reshape(4, 128, 256).transpose(1, 0, 2).astype(ml_dtypes.bfloat16)),
        "b3p": f32(np.asarray(inputs["b3"], np.float32).reshape(2, 128).T),
        "w4p": np.ascontiguousarray(W4.reshape(2, 128, 3).transpose(1, 0, 2).astype(ml_dtypes.bfloat16)),
        "b4p": f32(np.asarray(inputs["b4"], np.float32).reshape(3, 1)),
    }
    in_maps = []
    for c in range(NCORES):
        sl = slice(c * BS, (c + 1) * BS)
        m = dict(shared)
        m["obs_t"] = np.ascontiguousarray(np.asarray(inputs["obs"][sl], np.float32).transpose(2, 0, 1).astype(ml_dtypes.bfloat16))
        m["gt_t"] = np.ascontiguousarray(np.asarray(inputs["obs_gt"][sl], np.float32).transpose(2, 0, 1).astype(ml_dtypes.bfloat16))
        m["lat_t"] = f32(np.asarray(inputs["latent"][sl], np.float32).T)
        in_maps.append(m)

    res = run_bass_kernel_spmd(nc, in_maps, core_ids=list(range(NCORES)),
                               trace=TRACE)
    LAST = res

    parts = np.stack([r["partials"][0] for r in res.results]).astype(np.float64)
    s_max1 = parts[:, 0].sum()
    s_max2 = parts[:, 1].sum()
    s_gt2 = parts[:, 2].sum()
    s_est2 = parts[:, 3].sum()
    s_cross = parts[:, 4].sum()
    chm = (s_gt2 - 2.0 * s_max1) / (B * N) + (s_est2 - 2.0 * s_max2) / (B * N)
    l2 = (s_gt2 - 2.0 * s_cross + s_est2) / (B * N * 3)
    loss = 0.2 * chm + 0.8 * l2
    return np.asarray(loss, dtype=np.float32)



# revision 8
# speedup vs baseline: 1.1516x; 1.1516x over previous
"""Trainium2 Bass kernel for nn_DeepLatent loss (chamfer + L2 of a per-point MLP).

Strategy (8 cores, data-parallel over batch B=32 -> 4 samples/core):
  Per core, per sample s (channel-major layout: activations stored [C, Npoints]):
    h1 = relu(W1o.T @ obs^T + latbias)        latbias precomputed on host
    h2 = relu(W2.T @ h1 + b2)
    h3 = relu(W3.T @ h2 + b3)
    delta = W4.T @ h3                         est = obs + delta + b4
  Chamfer via a single augmented gram J = -d^2/2 (K=7 matmul):
    J[n,m] = gt_n . est_m - |est_m|^2/2 - |gt_n|^2/2
    lhsT rows: gt coords (3) | -0.5 (3) | -0.5*|gt|^2 (1, host-precomputed)
    rhs  rows: est coords (3) | est^2 (3) | ones (1)
    dir1 (per gt):  fused DVE tensor_tensor_reduce: J tile -> bf16 SBUF copy
                    + row-max accumulator in one pass.
    dir2 (per est): running elementwise bf16 max across the 8 gt tiles, then
                    8 PE transposes + one 3D-AP max-reduce.
  Sample s occupies partition band 32s (PE small-operand bases must be
  0/32/64/96 and lhsT/rhs must share a base).
  Scalar partials (max-sums, est^2 sum, cross sum) are combined on the host;
  sum(gt^2) is a pure input function computed on host.
"""

import ml_dtypes
import numpy as np
from contextlib import ExitStack

import concourse.bass as bass
import concourse.bacc as bacc
import concourse.mybir as mybir
import concourse.tile as tile
from concourse.bass_utils import run_bass_kernel_spmd

F32 = mybir.dt.float32
BF16 = mybir.dt.bfloat16
AX = mybir.AxisListType
OP = mybir.AluOpType
ACTF = mybir.ActivationFunctionType

B, N, L = 32, 1024, 256
NCORES = 8
BS = B // NCORES  # samples per core
NT = N // 128     # gram tiles per sample
NEG = -3.0e38

# test.py hooks
TRACE = False
LAST = None
DEBUG_DUMPS = False


def build_program():
    nc = bacc.Bacc()

    obs_d = nc.dram_tensor("obs_t", [128, N], BF16, kind="ExternalInput")[:]
    ainit_d = nc.dram_tensor("a_init", [128, N], BF16, kind="ExternalInput")[:]
    cinit_d = nc.dram_tensor("c_init", [128, N], BF16, kind="ExternalInput")[:]
    obs2_d = nc.dram_tensor("obs_t2", [32, N], BF16, kind="ExternalInput")[:]
    ainit2_d = nc.dram_tensor("a_init2", [32, N], BF16, kind="ExternalInput")[:]
    cinit2_d = nc.dram_tensor("c_init2", [32, N], BF16, kind="ExternalInput")[:]
    latb_d = nc.dram_tensor("latb_t", [128, 4, BS], F32, kind="ExternalInput")[:]
    W1od = nc.dram_tensor("w1o4", [128, 512], BF16, kind="ExternalInput")[:]
    eye3d = nc.dram_tensor("eye34", [128, 3], BF16, kind="ExternalInput")[:]
    eye128d = nc.dram_tensor("eye128", [128, 128], BF16, kind="ExternalInput")[:]
    W2d = nc.dram_tensor("w2p", [128, 4, 512], BF16, kind="ExternalInput")[:]
    b2d = nc.dram_tensor("b2p", [128, 4], F32, kind="ExternalInput")[:]
    W3d = nc.dram_tensor("w3p", [128, 4, 256], BF16, kind="ExternalInput")[:]
    b3d = nc.dram_tensor("b3p", [128, 2], F32, kind="ExternalInput")[:]
    W4d = nc.dram_tensor("w4p", [128, 2, 3], BF16, kind="ExternalInput")[:]
    b4d = nc.dram_tensor("b4p", [3, 1], F32, kind="ExternalInput")[:]
    out_d = nc.dram_tensor("partials", [1, 8], F32, kind="ExternalOutput")[:]
    if DEBUG_DUMPS:
        m1_d = nc.dram_tensor("dbg_m1", [128, NT * BS], F32, kind="ExternalOutput")[:]
        m2_d = nc.dram_tensor("dbg_m2", [128, NT * BS], F32, kind="ExternalOutput")[:]
        r0_d = nc.dram_tensor("dbg_r0", [128, N], BF16, kind="ExternalOutput")[:]
        r1_d = nc.dram_tensor("dbg_r1", [128, N], BF16, kind="ExternalOutput")[:]
        negt_d = nc.dram_tensor("dbg_negt", [128, N], BF16, kind="ExternalOutput")[:]

    with tile.TileContext(nc) as tc, ExitStack() as ctx:
        singles = ctx.enter_context(tc.tile_pool(name="singles", bufs=1))

        def fixed(shape, name, dtype=F32):
            return singles.tile(shape, dtype, tag=name, name=name)

        # ---------- fixed tiles ----------
        w1o4 = fixed([128, 512], "w1o4", BF16)
        eye34 = fixed([128, 3], "eye34", BF16)
        eye128 = fixed([128, 128], "eye128", BF16)
        latb = fixed([128, 4, BS], "latb")
        w2t = fixed([128, 4, 512], "w2t", BF16)
        w3t = fixed([128, 4, 256], "w3t", BF16)
        w4t = fixed([128, 2, 3], "w4t", BF16)
        b2t = fixed([128, 4], "b2t")
        b3t = fixed([128, 2], "b3t")
        b4p = fixed([3, 1], "b4p")
        obsA = fixed([128, N], "obsA", BF16)
        At = fixed([128, N], "At", BF16)
        Ct = fixed([128, N], "Ct", BF16)
        obsA2 = fixed([32, N], "obsA2", BF16)
        At2 = fixed([32, N], "At2", BF16)
        Ct2 = fixed([32, N], "Ct2", BF16)

        def bandof(s):
            return (obsA, At, Ct, 32 * s) if s < 3 else (obsA2, At2, Ct2, 0)
        Jc_ = [fixed([128, N], f"Jc{i}", BF16) for i in range(2)]
        R_ = [fixed([128, N], f"Rreg{i}", BF16) for i in range(2)]
        est_ = [fixed([3, N], f"est{i}", BF16) for i in range(2)]
        es2_ = [fixed([3, N], f"es2{i}", BF16) for i in range(2)]
        SES = fixed([3, BS], "SES")
        M1 = fixed([128, NT * BS], "M1")
        M2 = fixed([128, NT * BS], "M2")
        Ft = fixed([128, 8], "Ft")
        dumpx = fixed([128, N], "dumpx", BF16)
        ones_c = fixed([128, 1], "ones_c")
        outs = fixed([1, 8], "outs")

        h1p = ctx.enter_context(tc.tile_pool(name="h1", bufs=2))
        h2p = ctx.enter_context(tc.tile_pool(name="h2", bufs=2))
        h3p = ctx.enter_context(tc.tile_pool(name="h3", bufs=2))
        psA = ctx.enter_context(tc.tile_pool(name="psA", bufs=2, space="PSUM"))
        psG = ctx.enter_context(tc.tile_pool(name="psG", bufs=2, space="PSUM"))

        # ---------- startup ----------
        nc.sync.dma_start(out=w1o4, in_=W1od)
        nc.sync.dma_start(out=eye34, in_=eye3d)
        nc.sync.dma_start(out=eye128, in_=eye128d)
        nc.sync.dma_start(out=latb, in_=latb_d)
        nc.sync.dma_start(out=obsA, in_=obs_d)
        nc.sync.dma_start(out=At, in_=ainit_d)
        nc.sync.dma_start(out=Ct, in_=cinit_d)
        nc.sync.dma_start(out=obsA2, in_=obs2_d)
        nc.sync.dma_start(out=At2, in_=ainit2_d)
        nc.sync.dma_start(out=Ct2, in_=cinit2_d)
        nc.sync.dma_start(out=b2t, in_=b2d)
        nc.sync.dma_start(out=b3t, in_=b3d)
        nc.sync.dma_start(out=b4p, in_=b4d)
        nc.sync.dma_start(out=w2t, in_=W2d)
        nc.sync.dma_start(out=w3t, in_=W3d)
        nc.sync.dma_start(out=w4t, in_=W4d)
        nc.vector.memset(Ft, 0.0)
        nc.vector.memset(ones_c, 1.0)

        # ---------- per-sample gram rounds (generator; interleaved with next MLP) ----------
        ACT_TILES = (2, 5)

        def gram_rounds(s):
            R = R_[s % 2]
            _, Ats, Cts, base = bandof(s)
            for t in range(NT):
                gp = psG.tile([128, 1024], F32, tag="g", name=f"gp{s}_{t}")
                for j in range(2):
                    nc.tensor.matmul(
                        gp[:, 512 * j:512 * (j + 1)],
                        Ats[base:base + 7, 128 * t:128 * (t + 1)],
                        Cts[base:base + 7, 512 * j:512 * (j + 1)],
                        start=True, stop=True)
                col = M1[:, NT * s + t:NT * s + t + 1]
                if t in ACT_TILES:
                    # offload PSUM drain to the Scalar engine; DVE then runs
                    # in fast 2x bf16 mode out of SBUF
                    jc = Jc_[ACT_TILES.index(t) % 2]
                    nc.scalar.activation(jc, gp[:, :], ACTF.Copy)
                    src_ = jc
                else:
                    src_ = gp[:, :]
                nc.vector.tensor_reduce(out=col, in_=src_, axis=AX.X, op=OP.max)
                if t == 0:
                    nc.vector.tensor_copy(R, src_)
                else:
                    nc.vector.tensor_tensor(out=R, in0=src_, in1=R, op=OP.max)
                yield
            # dir2 finish: transpose running colmax R, then per-block row-max
            rt = psG.tile([128, NT, 128], BF16, tag="g", name=f"rt{s}")
            for k in range(NT):
                nc.tensor.transpose(rt[:, k, :], R[:, 128 * k:128 * (k + 1)],
                                    eye128)
            yield
            nc.vector.tensor_reduce(out=M2[:, NT * s:NT * (s + 1)],
                                    in_=rt[:, :, :], axis=AX.X, op=OP.max)
            yield

        def advance(it):
            if it is not None:
                next(it, None)

        # ---------- per-sample MLP ----------
        def mlp(s, hooks):
            obsAs, Ats, Cts, base = bandof(s)
            obsT = obsAs[base:base + 3, :]
            h1t = h1p.tile([128, 4, N], BF16, tag="h1", name=f"h1_{s}")
            for c in range(4):
                ps = psA.tile([128, N], F32, tag="a", name=f"l1ps{s}_{c}")
                for j in range(2):
                    nc.tensor.matmul(ps[:, 512 * j:512 * (j + 1)],
                                     w1o4[base:base + 3, 128 * c:128 * (c + 1)],
                                     obsT[:, 512 * j:512 * (j + 1)],
                                     start=True, stop=True)
                nc.scalar.activation(h1t[:, c, :], ps[:, :], ACTF.Relu,
                                     bias=latb[:, c, s:s + 1])
                advance(hooks)

            h2t = h2p.tile([128, 4, N], BF16, tag="h2", name=f"h2_{s}")
            for c in range(4):
                ps = psA.tile([128, N], F32, tag="a", name=f"l2ps{s}_{c}")
                for j in range(2):
                    for k in range(4):
                        nc.tensor.matmul(ps[:, 512 * j:512 * (j + 1)],
                                         w2t[:, k, 128 * c:128 * (c + 1)],
                                         h1t[:, k, 512 * j:512 * (j + 1)],
                                         start=(k == 0), stop=(k == 3))
                nc.scalar.activation(h2t[:, c, :], ps[:, :], ACTF.Relu,
                                     bias=b2t[:, c:c + 1])
                advance(hooks)

            h3t = h3p.tile([128, 2, N], BF16, tag="h3", name=f"h3_{s}")
            for c in range(2):
                ps = psA.tile([128, N], F32, tag="a", name=f"l3ps{s}_{c}")
                for j in range(2):
                    for k in range(4):
                        nc.tensor.matmul(ps[:, 512 * j:512 * (j + 1)],
                                         w3t[:, k, 128 * c:128 * (c + 1)],
                                         h2t[:, k, 512 * j:512 * (j + 1)],
                                         start=(k == 0), stop=(k == 3))
                nc.scalar.activation(h3t[:, c, :], ps[:, :], ACTF.Relu,
                                     bias=b3t[:, c:c + 1])
                advance(hooks)

            ps4 = psG.tile([128, 1024], F32, tag="g", name=f"l4ps{s}")
            for j in range(2):
                for k in range(2):
                    nc.tensor.matmul(ps4[0:3, 512 * j:512 * (j + 1)],
                                     w4t[:, k, :],
                                     h3t[:, k, 512 * j:512 * (j + 1)],
                                     start=(k == 0), stop=False)
                # obs folded into the same accumulation via identity rows
                nc.tensor.matmul(ps4[0:3, 512 * j:512 * (j + 1)],
                                 eye34[base:base + 3, :],
                                 obsT[:, 512 * j:512 * (j + 1)],
                                 start=False, stop=True)
            est, es2 = est_[s % 2], es2_[s % 2]
            nc.scalar.activation(est[:, :], ps4[0:3, :],
                                 ACTF.Identity, bias=b4p[:, 0:1])
            nc.scalar.activation(es2[:, :], est[:, :], ACTF.Square,
                                 accum_out=SES[:, s:s + 1])
            # move est / est^2 into the gram rhs band for this sample
            nc.gpsimd.dma_start(out=Cts[base:base + 3, :], in_=est[:, :])
            nc.gpsimd.dma_start(out=Cts[base + 3:base + 6, :], in_=es2[:, :])
            # cross term sum(gt*est) for the L2 loss
            crosscol = 4 if s < 3 else 5
            nc.vector.scalar_tensor_tensor(
                out=dumpx[base:base + 3, :], in0=Ats[base:base + 3, :],
                scalar=0.0, in1=Cts[base:base + 3, :], op0=OP.add, op1=OP.mult,
                accum_out=Ft[base:base + 3, crosscol:crosscol + 1])
            advance(hooks)

        pending = None
        for s in range(BS):
            mlp(s, pending)
            if pending is not None:
                for _ in pending:
                    pass
            pending = gram_rounds(s)
        if pending is not None:
            for _ in pending:
                pass

        # ---------- finale ----------
        nc.vector.tensor_reduce(out=Ft[:, 0:1], in_=M1[:, :], axis=AX.X, op=OP.add)
        nc.vector.tensor_reduce(out=Ft[:, 1:2], in_=M2[:, :], axis=AX.X, op=OP.add)
        nc.vector.tensor_reduce(out=Ft[0:3, 3:4], in_=SES[:, :], axis=AX.X, op=OP.add)

        fps = psG.tile([128, 1024], F32, tag="g", name="fps")
        nc.tensor.matmul(fps[0:1, 0:8], ones_c[:, :], Ft[:, :],
                         start=True, stop=True)
        nc.scalar.activation(outs[:, :], fps[0:1, 0:8], ACTF.Copy)
        nc.sync.dma_start(out=out_d, in_=outs)
        if DEBUG_DUMPS:
            nc.sync.dma_start(out=m1_d, in_=M1)
            nc.sync.dma_start(out=m2_d, in_=M2)
            nc.sync.dma_start(out=r0_d, in_=R_[0])
            nc.sync.dma_start(out=r1_d, in_=R_[1])
            nc.sync.dma_start(out=negt_d, in_=NEGT)

    nc.compile()
    return nc


_program_cache = []


def kernel(**inputs):
    global LAST
    if not _program_cache:
        _program_cache.append(build_program())
    nc = _program_cache[0]

    def f32(x):
        return np.ascontiguousarray(np.asarray(x, dtype=np.float32))

    W1 = np.asarray(inputs["W1"], np.float32)
    W2 = np.asarray(inputs["W2"], np.float32)
    W3 = np.asarray(inputs["W3"], np.float32)
    W4 = np.asarray(inputs["W4"], np.float32)
    b1 = np.asarray(inputs["b1"], np.float32)
    latent = np.asarray(inputs["latent"], np.float32)
    obs = np.asarray(inputs["obs"], np.float32)
    gt = np.asarray(inputs["obs_gt"], np.float32)

    w1o4 = np.zeros((128, 512), np.float32)
    eye34 = np.zeros((128, 3), np.float32)
    for s in range(3):
        w1o4[32 * s:32 * s + 3] = W1[0:3, :]
        eye34[32 * s:32 * s + 3] = np.eye(3, dtype=np.float32)

    shared = {
        "w1o4": np.ascontiguousarray(w1o4.astype(ml_dtypes.bfloat16)),
        "eye34": np.ascontiguousarray(eye34.astype(ml_dtypes.bfloat16)),
        "eye128": np.eye(128, dtype=ml_dtypes.bfloat16),
        "w2p": np.ascontiguousarray(W2.reshape(4, 128, 512).transpose(1, 0, 2).astype(ml_dtypes.bfloat16)),
        "b2p": f32(np.asarray(inputs["b2"], np.float32).reshape(4, 128).T),
        "w3p": np.ascontiguousarray(W3.reshape(4, 128, 256).transpose(1, 0, 2).astype(ml_dtypes.bfloat16)),
        "b3p": f32(np.asarray(inputs["b3"], np.float32).reshape(2, 128).T),
        "w4p": np.ascontiguousarray(W4.reshape(2, 128, 3).transpose(1, 0, 2).astype(ml_dtypes.bfloat16)),
        "b4p": f32(np.asarray(inputs["b4"], np.float32).reshape(3, 1)),
    }
    # latent bias: per-sample vector, tiny -> precompute on host
    lb_all = latent @ W1[3:, :] + b1  # [B, 512]

    in_maps = []
    for c in range(NCORES):
        sl = slice(c * BS, (c + 1) * BS)
        m = dict(shared)
        m["latb_t"] = f32(lb_all[sl].reshape(BS, 4, 128).transpose(2, 1, 0))
        obsc = obs[sl]                    # [BS, N, 3]
        gtc = gt[sl]                      # [BS, N, 3]
        g2 = (gtc * gtc).sum(-1)          # [BS, N]
        O = np.zeros((160, N), np.float32)
        A = np.zeros((160, N), np.float32)
        C = np.zeros((160, N), np.float32)
        for s in range(BS):
            r = 32 * s if s < 3 else 128
            O[r:r + 3] = obsc[s].T
            A[r:r + 3] = gtc[s].T
            A[r + 3:r + 6] = -0.5
            A[r + 6] = -0.5 * g2[s]
            C[r + 6] = 1.0
        m["obs_t"] = np.ascontiguousarray(O[:128].astype(ml_dtypes.bfloat16))
        m["a_init"] = np.ascontiguousarray(A[:128].astype(ml_dtypes.bfloat16))
        m["c_init"] = np.ascontiguousarray(C[:128].astype(ml_dtypes.bfloat16))
        m["obs_t2"] = np.ascontiguousarray(O[128:].astype(ml_dtypes.bfloat16))
        m["a_init2"] = np.ascontiguousarray(A[128:].astype(ml_dtypes.bfloat16))
        m["c_init2"] = np.ascontiguousarray(C[128:].astype(ml_dtypes.bfloat16))
        in_maps.append(m)

    res = run_bass_kernel_spmd(nc, in_maps, core_ids=list(range(NCORES)),
                               trace=TRACE)
    LAST = res

    parts = np.stack([r["partials"][0] for r in res.results]).astype(np.float64)
    s_maxJ1 = parts[:, 0].sum()
    s_maxJ2 = parts[:, 1].sum()
    s_est2 = parts[:, 3].sum()
    s_cross = parts[:, 4].sum() + parts[:, 5].sum()
    s_gt2 = float((gt.astype(np.float64) ** 2).sum())
    chm = (-2.0 * s_maxJ1 - 2.0 * s_maxJ2) / (B * N)
    l2 = (s_gt2 - 2.0 * s_cross + s_est2) / (B * N * 3)
    loss = 0.2 * chm + 0.8 * l2
    return np.asarray(loss, dtype=np.float32)


# revision 10
# speedup vs baseline: 1.2747x; 1.1069x over previous
"""Trainium2 Bass kernel for nn_DeepLatent loss (chamfer + L2 of a per-point MLP).

Strategy (8 cores, data-parallel over batch B=32 -> 4 samples/core):
  Per core, per sample s (channel-major layout: activations stored [C, Npoints]):
    h1 = relu(W1o.T @ obs^T + latbias)        latbias precomputed on host
    h2 = relu(W2.T @ h1 + b2)
    h3 = relu(W3.T @ h2 + b3)
    delta = W4.T @ h3                         est = obs + delta + b4
  Chamfer via a single augmented gram J = -d^2/2 (K=7 matmul):
    J[n,m] = gt_n . est_m - |est_m|^2/2 - |gt_n|^2/2
    lhsT rows: gt coords (3) | -0.5 (3) | -0.5*|gt|^2 (1, host-precomputed)
    rhs  rows: est coords (3) | est^2 (3) | ones (1)
    dir1 (per gt):  fused DVE tensor_tensor_reduce: J tile -> bf16 SBUF copy
                    + row-max accumulator in one pass.
    dir2 (per est): running elementwise bf16 max across the 8 gt tiles, then
                    8 PE transposes + one 3D-AP max-reduce.
  Sample s occupies partition band 32s (PE small-operand bases must be
  0/32/64/96 and lhsT/rhs must share a base).
  Scalar partials (max-sums, est^2 sum, cross sum) are combined on the host;
  sum(gt^2) is a pure input function computed on host.
"""

import ml_dtypes
import numpy as np
from contextlib import ExitStack

import concourse.bass as bass
import concourse.bacc as bacc
import concourse.mybir as mybir
import concourse.tile as tile
from concourse.bass_utils import run_bass_kernel_spmd

F32 = mybir.dt.float32
BF16 = mybir.dt.bfloat16
AX = mybir.AxisListType
OP = mybir.AluOpType
ACTF = mybir.ActivationFunctionType

B, N, L = 32, 1024, 256
NCORES = 8
BS = B // NCORES  # samples per core
NT = N // 128     # gram tiles per sample
NEG = -3.0e38

# test.py hooks
TRACE = False
LAST = None
DEBUG_DUMPS = False


def build_program():
    nc = bacc.Bacc()

    obs_d = nc.dram_tensor("obs_t", [128, N], BF16, kind="ExternalInput")[:]
    ainit_d = nc.dram_tensor("a_init", [128, N], BF16, kind="ExternalInput")[:]
    cinit_d = nc.dram_tensor("c_init", [128, N], BF16, kind="ExternalInput")[:]
    obs2_d = nc.dram_tensor("obs_t2", [32, N], BF16, kind="ExternalInput")[:]
    ainit2_d = nc.dram_tensor("a_init2", [32, N], BF16, kind="ExternalInput")[:]
    cinit2_d = nc.dram_tensor("c_init2", [32, N], BF16, kind="ExternalInput")[:]
    latb_d = nc.dram_tensor("latb_t", [128, 4, BS], F32, kind="ExternalInput")[:]
    W1od = nc.dram_tensor("w1o4", [128, 512], BF16, kind="ExternalInput")[:]
    eye3d = nc.dram_tensor("eye34", [128, 3], BF16, kind="ExternalInput")[:]
    eye128d = nc.dram_tensor("eye128", [128, 128], BF16, kind="ExternalInput")[:]
    W2d = nc.dram_tensor("w2p", [128, 4, 512], BF16, kind="ExternalInput")[:]
    b2d = nc.dram_tensor("b2p", [128, 4], F32, kind="ExternalInput")[:]
    W3d = nc.dram_tensor("w3p", [128, 4, 256], BF16, kind="ExternalInput")[:]
    b3d = nc.dram_tensor("b3p", [128, 2], F32, kind="ExternalInput")[:]
    W4d = nc.dram_tensor("w4p", [128, 2, 3], BF16, kind="ExternalInput")[:]
    b4d = nc.dram_tensor("b4p", [3, 1], F32, kind="ExternalInput")[:]
    out_d = nc.dram_tensor("partials", [1, 8], F32, kind="ExternalOutput")[:]
    if DEBUG_DUMPS:
        m1_d = nc.dram_tensor("dbg_m1", [128, NT * BS], F32, kind="ExternalOutput")[:]
        m2_d = nc.dram_tensor("dbg_m2", [128, NT * BS], F32, kind="ExternalOutput")[:]
        r0_d = nc.dram_tensor("dbg_r0", [128, N], BF16, kind="ExternalOutput")[:]
        r1_d = nc.dram_tensor("dbg_r1", [128, N], BF16, kind="ExternalOutput")[:]
        negt_d = nc.dram_tensor("dbg_negt", [128, N], BF16, kind="ExternalOutput")[:]

    with tile.TileContext(nc) as tc, ExitStack() as ctx:
        singles = ctx.enter_context(tc.tile_pool(name="singles", bufs=1))

        def fixed(shape, name, dtype=F32):
            return singles.tile(shape, dtype, tag=name, name=name)

        # ---------- fixed tiles ----------
        w1o4 = fixed([128, 512], "w1o4", BF16)
        eye34 = fixed([128, 3], "eye34", BF16)
        eye128 = fixed([128, 128], "eye128", BF16)
        latb = fixed([128, 4, BS], "latb")
        w2t = fixed([128, 4, 512], "w2t", BF16)
        w3t = fixed([128, 4, 256], "w3t", BF16)
        w4t = fixed([128, 2, 3], "w4t", BF16)
        b2t = fixed([128, 4], "b2t")
        b3t = fixed([128, 2], "b3t")
        b4p = fixed([3, 1], "b4p")
        obsA = fixed([128, N], "obsA", BF16)
        At = fixed([128, N], "At", BF16)
        Ct = fixed([128, N], "Ct", BF16)
        obsA2 = fixed([32, N], "obsA2", BF16)
        At2 = fixed([32, N], "At2", BF16)
        Ct2 = fixed([32, N], "Ct2", BF16)

        def bandof(s):
            return (obsA, At, Ct, 32 * s) if s < 3 else (obsA2, At2, Ct2, 0)
        Jc_ = [fixed([128, N], f"Jc{i}", BF16) for i in range(2)]
        R_ = [fixed([128, N], f"Rreg{i}", BF16) for i in range(2)]
        est_ = [fixed([3, N], f"est{i}", BF16) for i in range(2)]
        es2_ = [fixed([3, N], f"es2{i}", BF16) for i in range(2)]
        SES = fixed([3, BS], "SES")
        M1 = fixed([128, NT * BS], "M1")
        M2 = fixed([128, NT * BS], "M2")
        Ft = fixed([128, 8], "Ft")
        dumpx = fixed([128, N], "dumpx", BF16)
        ones_c = fixed([128, 1], "ones_c")
        outs = fixed([1, 8], "outs")

        h1p = ctx.enter_context(tc.tile_pool(name="h1", bufs=2))
        h2p = ctx.enter_context(tc.tile_pool(name="h2", bufs=2))
        h3p = ctx.enter_context(tc.tile_pool(name="h3", bufs=2))
        psA = ctx.enter_context(tc.tile_pool(name="psA", bufs=2, space="PSUM"))
        psG = ctx.enter_context(tc.tile_pool(name="psG", bufs=2, space="PSUM"))

        # ---------- startup ----------
        # spread triggers across queues: each dma_start costs ~0.6us of its
        # issuing sequencer; sample-0 critical path (w1o4/latb/obsA) goes first
        nc.scalar.dma_start(out=w1o4, in_=W1od)
        nc.scalar.dma_start(out=latb, in_=latb_d)
        nc.scalar.dma_start(out=obsA, in_=obs_d)
        nc.scalar.dma_start(out=obsA2, in_=obs2_d)
        nc.gpsimd.dma_start(out=b2t, in_=b2d)
        nc.gpsimd.dma_start(out=b3t, in_=b3d)
        nc.gpsimd.dma_start(out=b4p, in_=b4d)
        nc.sync.dma_start(out=eye34, in_=eye3d)
        nc.sync.dma_start(out=eye128, in_=eye128d)
        nc.sync.dma_start(out=At, in_=ainit_d)
        nc.sync.dma_start(out=Ct, in_=cinit_d)
        nc.sync.dma_start(out=At2, in_=ainit2_d)
        nc.sync.dma_start(out=Ct2, in_=cinit2_d)
        nc.gpsimd.dma_start(out=w2t, in_=W2d)
        nc.gpsimd.dma_start(out=w3t, in_=W3d)
        nc.gpsimd.dma_start(out=w4t, in_=W4d)
        nc.vector.memset(Ft, 0.0)
        nc.vector.memset(ones_c, 1.0)

        # ---------- per-sample gram rounds (generator; interleaved with next MLP) ----------
        def gram_rounds(s):
            # tiles drained by an ACT copy (then DVE runs 2x out of SBUF);
            # last sample: everything on ACT/SBUF -- nothing overlaps the tail
            act_tiles = (0, 3, 6) if s < BS - 1 else tuple(range(NT))
            R = R_[s % 2]
            _, Ats, Cts, base = bandof(s)
            nact = 0
            t0src = None
            for t in range(NT):
                gp = psG.tile([128, 1024], F32, tag="g", name=f"gp{s}_{t}")
                for j in range(2):
                    nc.tensor.matmul(
                        gp[:, 512 * j:512 * (j + 1)],
                        Ats[base:base + 7, 128 * t:128 * (t + 1)],
                        Cts[base:base + 7, 512 * j:512 * (j + 1)],
                        start=True, stop=True)
                col = M1[:, NT * s + t:NT * s + t + 1]
                if t in act_tiles:
                    jc = Jc_[nact % 2]
                    nact += 1
                    nc.scalar.activation(jc, gp[:, :], ACTF.Copy)
                    src_ = jc
                else:
                    src_ = gp[:, :]
                nc.vector.tensor_reduce(out=col, in_=src_, axis=AX.X, op=OP.max)
                if t == 0:
                    t0src = src_  # defer R init: first TT combines t0 and t1
                elif t == 1:
                    nc.vector.tensor_tensor(out=R, in0=src_, in1=t0src, op=OP.max)
                else:
                    nc.vector.tensor_tensor(out=R, in0=src_, in1=R, op=OP.max)
                yield
            # dir2 finish: transpose running colmax R, then per-block row-max
            rt = psG.tile([128, NT, 128], BF16, tag="g", name=f"rt{s}")
            for k in range(NT):
                nc.tensor.transpose(rt[:, k, :], R[:, 128 * k:128 * (k + 1)],
                                    eye128)
            yield
            nc.vector.tensor_reduce(out=M2[:, NT * s:NT * (s + 1)],
                                    in_=rt[:, :, :], axis=AX.X, op=OP.max)
            yield

        def advance(it):
            if it is not None:
                next(it, None)

        # ---------- per-sample MLP ----------
        def mlp(s, hooks):
            obsAs, Ats, Cts, base = bandof(s)
            obsT = obsAs[base:base + 3, :]
            h1t = h1p.tile([128, 4, N], BF16, tag="h1", name=f"h1_{s}")
            for c in range(4):
                ps = psA.tile([128, N], F32, tag="a", name=f"l1ps{s}_{c}")
                for j in range(2):
                    nc.tensor.matmul(ps[:, 512 * j:512 * (j + 1)],
                                     w1o4[base:base + 3, 128 * c:128 * (c + 1)],
                                     obsT[:, 512 * j:512 * (j + 1)],
                                     start=True, stop=True)
                nc.scalar.activation(h1t[:, c, :], ps[:, :], ACTF.Relu,
                                     bias=latb[:, c, s:s + 1])
                advance(hooks)

            h2t = h2p.tile([128, 4, N], BF16, tag="h2", name=f"h2_{s}")
            for c in range(4):
                ps = psA.tile([128, N], F32, tag="a", name=f"l2ps{s}_{c}")
                for j in range(2):
                    for k in range(4):
                        nc.tensor.matmul(ps[:, 512 * j:512 * (j + 1)],
                                         w2t[:, k, 128 * c:128 * (c + 1)],
                                         h1t[:, k, 512 * j:512 * (j + 1)],
                                         start=(k == 0), stop=(k == 3))
                nc.scalar.activation(h2t[:, c, :], ps[:, :], ACTF.Relu,
                                     bias=b2t[:, c:c + 1])
                advance(hooks)

            h3t = h3p.tile([128, 2, N], BF16, tag="h3", name=f"h3_{s}")
            for c in range(2):
                ps = psA.tile([128, N], F32, tag="a", name=f"l3ps{s}_{c}")
                for j in range(2):
                    for k in range(4):
                        nc.tensor.matmul(ps[:, 512 * j:512 * (j + 1)],
                                         w3t[:, k, 128 * c:128 * (c + 1)],
                                         h2t[:, k, 512 * j:512 * (j + 1)],
                                         start=(k == 0), stop=(k == 3))
                nc.scalar.activation(h3t[:, c, :], ps[:, :], ACTF.Relu,
                                     bias=b3t[:, c:c + 1])
                advance(hooks)

            ps4 = psA.tile([128, N], F32, tag="a", name=f"l4ps{s}")
            for j in range(2):
                for k in range(2):
                    nc.tensor.matmul(ps4[0:3, 512 * j:512 * (j + 1)],
                                     w4t[:, k, :],
                                     h3t[:, k, 512 * j:512 * (j + 1)],
                                     start=(k == 0), stop=False)
                # obs folded into the same accumulation via identity rows
                nc.tensor.matmul(ps4[0:3, 512 * j:512 * (j + 1)],
                                 eye34[base:base + 3, :],
                                 obsT[:, 512 * j:512 * (j + 1)],
                                 start=False, stop=True)
            est, es2 = est_[s % 2], es2_[s % 2]
            nc.scalar.activation(est[:, :], ps4[0:3, :],
                                 ACTF.Identity, bias=b4p[:, 0:1])
            nc.scalar.activation(es2[:, :], est[:, :], ACTF.Square,
                                 accum_out=SES[:, s:s + 1])
            # move est / est^2 into the gram rhs band for this sample
            nc.gpsimd.dma_start(out=Cts[base:base + 3, :], in_=est[:, :])
            nc.gpsimd.dma_start(out=Cts[base + 3:base + 6, :], in_=es2[:, :])
            # cross term sum(gt*est) for the L2 loss
            crosscol = 4 if s < 3 else 5
            nc.vector.scalar_tensor_tensor(
                out=dumpx[base:base + 3, :], in0=Ats[base:base + 3, :],
                scalar=0.0, in1=Cts[base:base + 3, :], op0=OP.add, op1=OP.mult,
                accum_out=Ft[base:base + 3, crosscol:crosscol + 1])
            advance(hooks)

        pending = None
        for s in range(BS):
            mlp(s, pending)
            if pending is not None:
                for _ in pending:
                    pass
            pending = gram_rounds(s)
        if pending is not None:
            for _ in pending:
                pass

        # ---------- finale ----------
        nc.vector.tensor_reduce(out=Ft[:, 0:1], in_=M1[:, :], axis=AX.X, op=OP.add)
        nc.vector.tensor_reduce(out=Ft[:, 1:2], in_=M2[:, :], axis=AX.X, op=OP.add)
        nc.vector.tensor_reduce(out=Ft[0:3, 3:4], in_=SES[:, :], axis=AX.X, op=OP.add)

        fps = psG.tile([128, 1024], F32, tag="g", name="fps")
        nc.tensor.matmul(fps[0:1, 0:8], ones_c[:, :], Ft[:, :],
                         start=True, stop=True)
        nc.scalar.activation(outs[:, :], fps[0:1, 0:8], ACTF.Copy)
        nc.sync.dma_start(out=out_d, in_=outs)
        if DEBUG_DUMPS:
            nc.sync.dma_start(out=m1_d, in_=M1)
            nc.sync.dma_start(out=m2_d, in_=M2)
            nc.sync.dma_start(out=r0_d, in_=R_[0])
            nc.sync.dma_start(out=r1_d, in_=R_[1])
            nc.sync.dma_start(out=negt_d, in_=NEGT)

    nc.compile()
    return nc


_program_cache = []


def kernel(**inputs):
    global LAST
    if not _program_cache:
        _program_cache.append(build_program())
    nc = _program_cache[0]

    def f32(x):
        return np.ascontiguousarray(np.asarray(x, dtype=np.float32))

    W1 = np.asarray(inputs["W1"], np.float32)
    W2 = np.asarray(inputs["W2"], np.float32)
    W3 = np.asarray(inputs["W3"], np.float32)
    W4 = np.asarray(inputs["W4"], np.float32)
    b1 = np.asarray(inputs["b1"], np.float32)
    latent = np.asarray(inputs["latent"], np.float32)
    obs = np.asarray(inputs["obs"], np.float32)
    gt = np.asarray(inputs["obs_gt"], np.float32)

    w1o4 = np.zeros((128, 512), np.float32)
    eye34 = np.zeros((128, 3), np.float32)
    for s in range(3):
        w1o4[32 * s:32 * s + 3] = W1[0:3, :]
        eye34[32 * s:32 * s + 3] = np.eye(3, dtype=np.float32)

    shared = {
        "w1o4": np.ascontiguousarray(w1o4.astype(ml_dtypes.bfloat16)),
        "eye34": np.ascontiguousarray(eye34.astype(ml_dtypes.bfloat16)),
        "eye128": np.eye(128, dtype=ml_dtypes.bfloat16),
        "w2p": np.ascontiguousarray(W2.reshape(4, 128, 512).transpose(1, 0, 2).astype(ml_dtypes.bfloat16)),
        "b2p": f32(np.asarray(inputs["b2"], np.float32).reshape(4, 128).T),
        "w3p": np.ascontiguousarray(W3.reshape(4, 128, 256).transpose(1, 0, 2).astype(ml_dtypes.bfloat16)),
        "b3p": f32(np.asarray(inputs["b3"], np.float32).reshape(2, 128).T),
        "w4p": np.ascontiguousarray(W4.reshape(2, 128, 3).transpose(1, 0, 2).astype(ml_dtypes.bfloat16)),
        "b4p": f32(np.asarray(inputs["b4"], np.float32).reshape(3, 1)),
    }
    # latent bias: per-sample vector, tiny -> precompute on host
    lb_all = latent @ W1[3:, :] + b1  # [B, 512]

    in_maps = []
    for c in range(NCORES):
        sl = slice(c * BS, (c + 1) * BS)
        m = dict(shared)
        m["latb_t"] = f32(lb_all[sl].reshape(BS, 4, 128).transpose(2, 1, 0))
        obsc = obs[sl]                    # [BS, N, 3]
        gtc = gt[sl]                      # [BS, N, 3]
        g2 = (gtc * gtc).sum(-1)          # [BS, N]
        O = np.zeros((160, N), np.float32)
        A = np.zeros((160, N), np.float32)
        C = np.zeros((160, N), np.float32)
        for s in range(BS):
            r = 32 * s if s < 3 else 128
            O[r:r + 3] = obsc[s].T
            A[r:r + 3] = gtc[s].T
            A[r + 3:r + 6] = -0.5
            A[r + 6] = -0.5 * g2[s]
            C[r + 6] = 1.0
        m["obs_t"] = np.ascontiguousarray(O[:128].astype(ml_dtypes.bfloat16))
        m["a_init"] = np.ascontiguousarray(A[:128].astype(ml_dtypes.bfloat16))
        m["c_init"] = np.ascontiguousarray(C[:128].astype(ml_dtypes.bfloat16))
        m["obs_t2"] = np.ascontiguousarray(O[128:].astype(ml_dtypes.bfloat16))
        m["a_init2"] = np.ascontiguousarray(A[128:].astype(ml_dtypes.bfloat16))
        m["c_init2"] = np.ascontiguousarray(C[128:].astype(ml_dtypes.bfloat16))
        in_maps.append(m)

    res = run_bass_kernel_spmd(nc, in_maps, core_ids=list(range(NCORES)),
                               trace=TRACE)
    LAST = res

    parts = np.stack([r["partials"][0] for r in res.results]).astype(np.float64)
    s_maxJ1 = parts[:, 0].sum()
    s_maxJ2 = parts[:, 1].sum()
    s_est2 = parts[:, 3].sum()
    s_cross = parts[:, 4].sum() + parts[:, 5].sum()
    s_gt2 = float((gt.astype(np.float64) ** 2).sum())
    chm = (-2.0 * s_maxJ1 - 2.0 * s_maxJ2) / (B * N)
    l2 = (s_gt2 - 2.0 * s_cross + s_est2) / (B * N * 3)
    loss = 0.2 * chm + 0.8 * l2
    return np.asarray(loss, dtype=np.float32)
